# revision 1
# baseline (speedup 1.0000x reference)
"""Multi-head self-attention (B=8, S=1024, E=1024, H=16) on 8 TRN2 cores.

Sharding: data-parallel on batch — core i computes batch i, all 16 heads.
Device computes pure causal attention (bias folded into q/k/v); rows q >= l[b]
are zeroed on the host (causal & q<l implies k<l, so the padding mask is
redundant for valid rows).
"""

import sys

sys.path.insert(0, "/opt/trn_rl_repo")

import numpy as np
import ml_dtypes

import concourse.bass as bass
import concourse.bacc as bacc
import concourse.mybir as mybir
import concourse.tile as tile
from concourse.bass import ds, ts
from concourse.bass_utils import run_bass_kernel_spmd

P = 128
B, S, E, H = 8, 1024, 1024, 16
DH = E // H  # 64
NT = S // P  # 8
F32 = mybir.dt.float32
BF16 = mybir.dt.bfloat16
F32R = mybir.dt.float32r

_cached = None


def _build_program():
    nc = bacc.Bacc(None, target_bir_lowering=False)

    xT = nc.dram_tensor("xT", [E, S], BF16, kind="ExternalInput")[:]
    wT = nc.dram_tensor("wT", [E, 3 * E], BF16, kind="ExternalInput")[:]
    bqk = nc.dram_tensor("bqk", [P, 16], F32, kind="ExternalInput")[:]
    bv = nc.dram_tensor("bv", [1, E], BF16, kind="ExternalInput")[:]
    cm = nc.dram_tensor("cm", [P, P], BF16, kind="ExternalInput")[:]
    ones = nc.dram_tensor("ones", [1, P], BF16, kind="ExternalInput")[:]
    o = nc.dram_tensor("o", [S, E], F32, kind="ExternalOutput")[:]

    with tile.TileContext(nc) as tc:
        from contextlib import ExitStack

        with ExitStack() as ctx:
            sb = ctx.enter_context(tc.tile_pool(name="sb", bufs=1))
            xT_sb = sb.tile([P, NT, S], BF16)       # [e_p, e_t, s]
            qkT_sb = sb.tile([P, 16, S], BF16)      # [j_p, j_t, s] (8 Q tiles, 8 K tiles)
            vp_sb = sb.tile([P, NT, H, DH + 1], BF16)  # [s_p, s_t, h, d] + ones col
            out_sb = sb.tile([P, NT, E], F32)       # [q_p, t_q, j]
            bqk_sb = sb.tile([P, 16], F32)
            bv_sb = sb.tile([1, E], BF16)
            cm_sb = sb.tile([P, P], BF16)
            ones_sb = sb.tile([1, P], BF16)

            wblk_pool = ctx.enter_context(tc.tile_pool(name="wblk", bufs=3))
            qk_psum = ctx.enter_context(
                tc.tile_pool(name="qk_psum", bufs=2, space="PSUM"))

            for e_t in range(NT):
                nc.sync.dma_start(
                    out=xT_sb[:, e_t, :], in_=xT[ds(e_t * P, P), :])
            nc.sync.dma_start(out=bqk_sb, in_=bqk)
            nc.sync.dma_start(out=bv_sb, in_=bv)
            nc.sync.dma_start(out=cm_sb, in_=cm)
            nc.sync.dma_start(out=ones_sb, in_=ones)
            nc.vector.memset(vp_sb[:, :, :, DH : DH + 1], 1.0)

            def emit_qk(j_t):
                # qkT_sb[:, j_t, :] = (W_row_block @ x^T + bias), cast bf16
                wblk = wblk_pool.tile([P, NT, P], BF16)
                nc.sync.dma_start(
                    out=wblk,
                    in_=wT[:, ds(j_t * P, P)].rearrange("(t p) j -> p t j", p=P))
                for s_half in range(2):
                    ps = qk_psum.tile([P, 512], F32)
                    for e_t in range(NT):
                        nc.tensor.matmul(
                            ps,
                            lhsT=wblk[:, e_t, :],
                            rhs=xT_sb[:, e_t, ds(s_half * 512, 512)],
                            start=(e_t == 0),
                            stop=(e_t == NT - 1))
                    nc.scalar.activation(
                        out=qkT_sb[:, j_t, ds(s_half * 512, 512)],
                        in_=ps,
                        func=mybir.ActivationFunctionType.Identity,
                        bias=bqk_sb[:, ds(j_t, 1)],
                        scale=1.0)

            def emit_v(jv_half, wv_pool):
                # vp_sb[:, s_t, 8*jv_half:+8, 0:64] = x @ W_v_cols + bias
                wv = wv_pool.tile([P, NT, 512], BF16, name="wv")
                nc.sync.dma_start(
                    out=wv,
                    in_=wT[:, ds(2 * E + jv_half * 512, 512)].rearrange(
                        "(t p) j -> p t j", p=P))
                for s_t in range(NT):
                    ps = qk_psum.tile([P, 512], F32)
                    for e_t in range(NT):
                        nc.tensor.matmul(
                            ps,
                            lhsT=xT_sb[:, e_t, ts(s_t, P)],
                            rhs=wv[:, e_t, :],
                            start=(e_t == 0),
                            stop=False)
                    nc.tensor.matmul(
                        ps,
                        lhsT=ones_sb,
                        rhs=bv_sb[:, ds(jv_half * 512, 512)],
                        start=False,
                        stop=True)
                    nc.vector.tensor_copy(
                        out=vp_sb[:, s_t, ds(jv_half * 8, 8), 0:DH],
                        in_=ps.rearrange("p (h d) -> p h d", h=8))

            def emit_attn(hp):
                h0, h1 = 2 * hp, 2 * hp + 1
                eT = {h: eT_pool.tile([P, NT, S], BF16, name="eT")
                      for h in (h0, h1)}
                for t_k in range(NT):
                    q0 = t_k * P
                    if t_k < 4:
                        chunks = [(q0, 512 - q0), (512, 512)]
                    else:
                        chunks = [(q0, S - q0)]
                    for (c0, cn) in chunks:
                        for h, base in ((h0, 0), (h1, 64)):
                            ps = s_psum.tile([P, 512], F32)
                            nc.tensor.matmul(
                                ps[:, 0:cn],
                                lhsT=qkT_sb[base:base + 64, 8 + hp, ts(t_k, P)],
                                rhs=qkT_sb[base:base + 64, hp, ds(c0, cn)],
                                start=True,
                                stop=True)
                            nc.scalar.activation(
                                out=eT[h][:, t_k, ds(c0, cn)],
                                in_=ps[:, 0:cn],
                                func=mybir.ActivationFunctionType.Exp,
                                scale=1.0 / 32.0)
                    for h in (h0, h1):
                        nc.vector.tensor_mul(
                            eT[h][:, t_k, ds(q0, P)],
                            eT[h][:, t_k, ds(q0, P)],
                            cm_sb)
                for h in (h0, h1):
                    for t_q in range(NT):
                        po = o_psum.tile([P, 512], F32)
                        for t_k in range(t_q + 1):
                            nc.tensor.matmul(
                                po[:, 0:DH + 1],
                                lhsT=eT[h][:, t_k, ts(t_q, P)],
                                rhs=vp_sb[:, t_k, h, :],
                                start=(t_k == 0),
                                stop=(t_k == t_q))
                        rec = rec_pool.tile([P, 1], F32)
                        nc.vector.reciprocal(rec, po[:, DH:DH + 1])
                        nc.vector.tensor_scalar_mul(
                            out_sb[:, t_q, ds(h * DH, DH)],
                            po[:, 0:DH],
                            rec)

            # Emission schedule: keep PE fed, overlap phase1 with phase2.
            emit_qk(0)       # Q pair 0
            emit_qk(8)       # K pair 0
            with tc.tile_pool(name="wv", bufs=2) as wv_pool:
                emit_v(0, wv_pool)   # heads 0-7
                emit_v(1, wv_pool)   # heads 8-15
            eT_pool = ctx.enter_context(tc.tile_pool(name="eT", bufs=3))
            rec_pool = ctx.enter_context(tc.tile_pool(name="rec", bufs=4))
            s_psum = ctx.enter_context(
                tc.tile_pool(name="s_psum", bufs=4, space="PSUM"))
            o_psum = ctx.enter_context(
                tc.tile_pool(name="o_psum", bufs=2, space="PSUM"))
            emit_qk(1)
            emit_qk(9)
            for hp in range(8):
                emit_attn(hp)
                if hp + 2 < 8:
                    emit_qk(hp + 2)
                    emit_qk(8 + hp + 2)
                for t_q in range(NT):
                    nc.sync.dma_start(
                        out=o[ts(t_q, P), ds(hp * P, P)],
                        in_=out_sb[:, t_q, ds(hp * P, P)])

    nc.compile()
    return nc


def _prepare_in_maps(x, l, W, b):
    wTc = np.ascontiguousarray(W.T.astype(ml_dtypes.bfloat16))
    bqk = np.ascontiguousarray(
        b[: 2 * E].astype(np.float32).reshape(16, P).T)
    bv = np.ascontiguousarray(
        b[2 * E :].astype(ml_dtypes.bfloat16).reshape(1, E))
    k_idx = np.arange(P)[:, None]
    q_idx = np.arange(P)[None, :]
    cm = (k_idx <= q_idx).astype(ml_dtypes.bfloat16)
    ones = np.ones((1, P), ml_dtypes.bfloat16)
    in_maps = []
    for bi in range(B):
        xTb = np.ascontiguousarray(x[bi].T.astype(ml_dtypes.bfloat16))
        in_maps.append(
            {"xT": xTb, "wT": wTc, "bqk": bqk, "bv": bv, "cm": cm,
             "ones": ones})
    return in_maps


def _run(x, l, W, b, trace=False):
    global _cached
    if _cached is None:
        _cached = _build_program()
    nc = _cached
    in_maps = _prepare_in_maps(x, l, W, b)
    res = run_bass_kernel_spmd(nc, in_maps, list(range(B)), trace=trace)
    out = np.stack([res.results[i]["o"] for i in range(B)]).astype(np.float32)
    lv = np.asarray(l).astype(np.int64)
    for bi in range(B):
        out[bi, int(lv[bi]) :, :] = 0.0
    return out, res.exec_time_ns


def kernel(x, l, W, b):
    out, _ = _run(x, l, W, b, trace=False)
    return out



# revision 12
# speedup vs baseline: 1.7448x; 1.7448x over previous
"""Multi-head self-attention (B=8, S=1024, E=1024, H=16) on 8 TRN2 cores.

Sharding: head-parallel with length clipping. Core c owns heads {2c, 2c+1}
for ALL batches; each batch b is clipped to nl_b = ceil(l_b/128) tiles of
128 sequence positions (causal attention means rows q < l_b never read
k >= l_b, and rows q >= l_b are zeroed on the host). Every core processes
the same multiset of per-batch lengths, so one SPMD program serves all
cores with perfectly balanced load; only the W/bias column slices differ
per core, and all x tiles are broadcast.

Per-core pipeline (per batch slot, nl tiles of 128):
  - QK projection bf16, outputs [j, s] with j = q|k feature groups of the
    2 heads; PSUM->SBUF copy with per-partition bias add on DVE.
  - V projection bf16 in [j, s] orientation (weights stationary, x moving
    512 wide), bias via ones-row matmul; transposed back to [s, j] with
    the XBAR DMA-transpose; ones column appended for the softmax
    denominator.
  - Scores per head as K^T tile x Q chunks (64-deep contraction); exp on
    Act; causal mask of the diagonal tile multiplied on the idle GpSimd
    (Pool) engine (SBUF-only op).
  - AV with PSUM column packing: [q,65] slots for 4 t_q share one bank,
    col 64 accumulating the denominator via the V ones-column.
  - Normalize with one reciprocal + one stride0-broadcast tensor_tensor
    per 4-t_q group on DVE; bf16 output staged and DMA'd per batch.
"""

import sys

sys.path.insert(0, "/opt/trn_rl_repo")

import numpy as np
import ml_dtypes

import concourse.bass as bass
import concourse.bacc as bacc
import concourse.mybir as mybir
import concourse.tile as tile
from concourse.bass import ds, ts, broadcast_tensor_aps
from concourse.bass_utils import run_bass_kernel_spmd

P = 128
B, S, E, H = 8, 1024, 1024, 16
DH = E // H  # 64
NT = S // P  # 8
F32 = mybir.dt.float32
BF16 = mybir.dt.bfloat16

_cached = {}


def _build_program(nls):
    nc = bacc.Bacc(None, target_bir_lowering=False)

    xb = [nc.dram_tensor(f"xb_{i}", [P, NT, nl * P], BF16,
                         kind="ExternalInput")[:] for i, nl in enumerate(nls)]
    wqk = nc.dram_tensor("wqk", [P, NT, 2, P], BF16, kind="ExternalInput")[:]
    wv = nc.dram_tensor("wv", [P, NT, P], BF16, kind="ExternalInput")[:]
    bqk = nc.dram_tensor("bqk", [P, 2], F32, kind="ExternalInput")[:]
    bv = nc.dram_tensor("bv", [1, P], BF16, kind="ExternalInput")[:]
    cm = nc.dram_tensor("cm", [P, P], BF16, kind="ExternalInput")[:]
    ident = nc.dram_tensor("ident", [P, P], BF16, kind="ExternalInput")[:]
    ones = nc.dram_tensor("ones", [1, 512], BF16, kind="ExternalInput")[:]
    total = sum(nl * P for nl in nls)
    o = nc.dram_tensor("o", [total, P], BF16, kind="ExternalOutput")[:]

    with tile.TileContext(nc) as tc:
        from contextlib import ExitStack

        with ExitStack() as ctx:
            sb = ctx.enter_context(tc.tile_pool(name="sb", bufs=1))
            wqk_sb = sb.tile([P, NT, 2, P], BF16)
            wv_sb = sb.tile([P, NT, P], BF16)
            bqk_sb = sb.tile([P, 2], F32)
            bv_sb = sb.tile([1, P], BF16)
            cm_sb = sb.tile([P, P], BF16)
            ident_sb = sb.tile([P, P], BF16)
            ones_sb = sb.tile([1, 512], BF16)

            nc.sync.dma_start(out=wqk_sb, in_=wqk)
            nc.sync.dma_start(out=wv_sb, in_=wv)

            xbp = ctx.enter_context(tc.tile_pool(name="xbp", bufs=2))

            def load_x(i, nl):
                # column-chunked so the first QK matmul starts early
                xbt = xbp.tile([P, NT, NT * P], BF16, name="xbt")
                for c0 in range(0, nl * P, 512):
                    cn = min(512, nl * P - c0)
                    eng = nc.sync if (c0 == 0 and i == 0) else nc.gpsimd
                    eng.dma_start(out=xbt[:, :, ds(c0, cn)],
                                  in_=xb[i][:, :, ds(c0, cn)])
                return xbt

            xtiles = {0: load_x(0, nls[0])}
            for t, src in [(bqk_sb, bqk), (bv_sb, bv), (cm_sb, cm),
                           (ident_sb, ident), (ones_sb, ones)]:
                nc.sync.dma_start(out=t, in_=src)

            qkp = ctx.enter_context(tc.tile_pool(name="qkp", bufs=2))
            vtp = ctx.enter_context(tc.tile_pool(name="vtp", bufs=2))
            vpp = ctx.enter_context(tc.tile_pool(name="vpp", bufs=2))
            etp = ctx.enter_context(tc.tile_pool(name="etp", bufs=2))
            otp = ctx.enter_context(tc.tile_pool(name="otp", bufs=2))
            rcp = ctx.enter_context(tc.tile_pool(name="rcp", bufs=4))
            proj_ps = ctx.enter_context(
                tc.tile_pool(name="proj_ps", bufs=2, space="PSUM"))
            tr_ps = ctx.enter_context(
                tc.tile_pool(name="tr_ps", bufs=1, space="PSUM"))
            sc_ps = ctx.enter_context(
                tc.tile_pool(name="sc_ps", bufs=2, space="PSUM"))
            av_ps = ctx.enter_context(
                tc.tile_pool(name="av_ps", bufs=3, space="PSUM"))

            rowbase = 0
            for i, nl in enumerate(nls):
                Si = nl * P
                xbt = xtiles.pop(i)
                if i + 1 < len(nls):
                    xtiles[i + 1] = load_x(i + 1, nls[i + 1])

                # --- QK projection (bf16): psum [j, s-chunk], j = 2 heads
                qk = qkp.tile([P, 2, NT * P], BF16, name="qk")
                for g in range(2):
                    for c0 in range(0, Si, 512):
                        cn = min(512, Si - c0)
                        ps = proj_ps.tile([P, 512], F32)
                        for et in range(NT):
                            nc.tensor.matmul(
                                ps[:, 0:cn],
                                lhsT=wqk_sb[:, et, g, :],
                                rhs=xbt[:, et, ds(c0, cn)],
                                start=(et == 0), stop=(et == NT - 1))
                        nc.vector.tensor_scalar_add(
                            out=qk[:, g, ds(c0, cn)], in0=ps[:, 0:cn],
                            scalar1=bqk_sb[:, ds(g, 1)])

                # --- V projection (bf16) in [j, s], then XBAR transpose
                vT = vtp.tile([P, NT * P], BF16, name="vT")
                for c0 in range(0, Si, 512):
                    cn = min(512, Si - c0)
                    ps = proj_ps.tile([P, 512], F32)
                    for et in range(NT):
                        nc.tensor.matmul(
                            ps[:, 0:cn],
                            lhsT=wv_sb[:, et, :],
                            rhs=xbt[:, et, ds(c0, cn)],
                            start=(et == 0), stop=False)
                    nc.tensor.matmul(ps[:, 0:cn], lhsT=bv_sb,
                                     rhs=ones_sb[:, 0:cn],
                                     start=False, stop=True)
                    nc.vector.tensor_copy(out=vT[:, ds(c0, cn)],
                                          in_=ps[:, 0:cn])
                vp = vpp.tile([P, NT, 2, DH + 1], BF16, name="vp")
                nc.gpsimd.memset(vp[:, 0:nl, :, DH:DH + 1], 1.0)
                for st0 in range(0, nl, 4):
                    gs = min(4, nl - st0)
                    pt = tr_ps.tile([P, 4, P], BF16)
                    for st in range(st0, st0 + gs):
                        nc.tensor.transpose(
                            pt[:, st - st0, :], vT[:, ts(st, P)], ident_sb)
                    nc.vector.tensor_copy(
                        out=vp[:, ds(st0, gs), :, 0:DH],
                        in_=pt[:, 0:gs, :].rearrange(
                            "p t (h d) -> p t h d", h=2))

                # --- Scores + exp per head; diag mask on Pool
                ets = []
                for h in range(2):
                    h0 = h * DH
                    eT = etp.tile([P, NT, NT * P], BF16, name="eT")
                    for t in range(nl):
                        c0 = t * P
                        while c0 < Si:
                            cn = min(512, Si - c0)
                            ps = sc_ps.tile([P, 512], F32)
                            nc.tensor.matmul(
                                ps[:, 0:cn],
                                lhsT=qk[h0:h0 + DH, 1, ts(t, P)],
                                rhs=qk[h0:h0 + DH, 0, ds(c0, cn)],
                                start=True, stop=True)
                            nc.scalar.activation(
                                out=eT[:, t, ds(c0, cn)], in_=ps[:, 0:cn],
                                func=mybir.ActivationFunctionType.Exp,
                                scale=1.0 / 32.0)
                            c0 += cn
                        nc.gpsimd.tensor_mul(
                            eT[:, t, ts(t, P)], eT[:, t, ts(t, P)], cm_sb)
                    ets.append(eT)

                # --- AV + normalize per head, 4 t_q per PSUM bank
                out_sb = otp.tile([P, NT, P], BF16, name="out_sb")
                for h in range(2):
                    eT = ets[h]
                    for tq0 in range(0, nl, 4):
                        g = min(4, nl - tq0)
                        po = av_ps.tile([P, 260], F32)
                        for tq in range(tq0, tq0 + g):
                            sl = tq - tq0
                            for tk in range(tq + 1):
                                nc.tensor.matmul(
                                    po[:, ds(sl * 65, DH + 1)],
                                    lhsT=eT[:, tk, ts(tq, P)],
                                    rhs=vp[:, tk, h, :],
                                    start=(tk == 0), stop=(tk == tq))
                        pot = po.rearrange("p (t c) -> p t c", c=65)
                        rec = rcp.tile([P, 4], F32, name="rec")
                        nc.vector.reciprocal(rec[:, 0:g], pot[:, 0:g, 64])
                        in0 = pot[:, 0:g, 0:DH]
                        in1 = rec[:, 0:g].rearrange("p (t o) -> p t o", o=1)
                        b0, b1 = broadcast_tensor_aps(in0, in1)
                        nc.vector.tensor_tensor(
                            out=out_sb[:, ds(tq0, g), ds(h * DH, DH)],
                            in0=b0, in1=b1, op=mybir.AluOpType.mult)

                nc.sync.dma_start(
                    out=o[ds(rowbase, Si), :].rearrange(
                        "(t p) c -> p t c", p=P),
                    in_=out_sb[:, 0:nl, :])
                rowbase += Si

    nc.compile()
    return nc


def _prepare(x, l, W, b):
    lv = np.asarray(l).astype(np.int64)
    nl = np.minimum((lv + P - 1) // P, NT).astype(np.int64)
    order = sorted(range(B), key=lambda i: -int(nl[i]))
    nls = tuple(int(nl[i]) for i in order)

    common = {}
    for i, bi in enumerate(order):
        n = nls[i]
        xT = np.ascontiguousarray(x[bi].T[:, 0:n * P])  # [E, n*128] f32
        xr = xT.reshape(NT, P, n * P)
        common[f"xb_{i}"] = np.ascontiguousarray(
            xr.transpose(1, 0, 2).astype(ml_dtypes.bfloat16))
    idx = np.arange(P)
    common["cm"] = np.ascontiguousarray(
        (idx[:, None] <= idx[None, :]).astype(ml_dtypes.bfloat16))
    common["ident"] = np.eye(P).astype(ml_dtypes.bfloat16)
    common["ones"] = np.ones((1, 512), ml_dtypes.bfloat16)

    in_maps = []
    for c in range(B):
        r0 = 2 * c * DH  # first feature row of this core's 2 heads
        wq = W[r0:r0 + P]             # [128, E]
        wk = W[E + r0:E + r0 + P]
        wvs = W[2 * E + r0:2 * E + r0 + P]
        wqk_c = np.stack([wq.T, wk.T], axis=1)      # [E, 2, 128]
        wqk_c = wqk_c.reshape(NT, P, 2, P).transpose(1, 0, 2, 3)
        wv_c = wvs.T.reshape(NT, P, P).transpose(1, 0, 2)
        m = dict(common)
        m["wqk"] = np.ascontiguousarray(wqk_c.astype(ml_dtypes.bfloat16))
        m["wv"] = np.ascontiguousarray(wv_c.astype(ml_dtypes.bfloat16))
        m["bqk"] = np.ascontiguousarray(
            np.stack([b[r0:r0 + P], b[E + r0:E + r0 + P]], axis=1)
            .astype(np.float32))
        m["bv"] = np.ascontiguousarray(
            b[2 * E + r0:2 * E + r0 + P].astype(ml_dtypes.bfloat16)
            .reshape(1, P))
        in_maps.append(m)
    return in_maps, order, nls


def _run(x, l, W, b, trace=False):
    x = np.asarray(x, dtype=np.float32)
    W = np.asarray(W, dtype=np.float32)
    b = np.asarray(b, dtype=np.float32)
    in_maps, order, nls = _prepare(x, l, W, b)
    if nls not in _cached:
        _cached[nls] = _build_program(nls)
    nc = _cached[nls]
    res = run_bass_kernel_spmd(nc, in_maps, list(range(B)), trace=trace)

    lv = np.asarray(l).astype(np.int64)
    out = np.zeros((B, S, E), dtype=np.float32)
    for c in range(B):
        oc = np.asarray(res.results[c]["o"]).astype(np.float32)
        rowbase = 0
        for i, bi in enumerate(order):
            n = nls[i]
            lb = int(lv[bi])
            rows = min(lb, n * P)
            out[bi, 0:rows, P * c:P * (c + 1)] = oc[rowbase:rowbase + rows]
            rowbase += n * P
    return out, res.exec_time_ns


def kernel(x, l, W, b):
    out, _ = _run(x, l, W, b, trace=False)
    return out


# revision 17
# speedup vs baseline: 1.9072x; 1.0931x over previous
"""Multi-head self-attention (B=8, S=1024, E=1024, H=16) on 8 TRN2 cores.

Sharding: head-parallel with length clipping. Core c owns heads {2c, 2c+1}
for ALL batches; each batch b is clipped to nl_b = ceil(l_b/128) tiles of
128 sequence positions (causal attention means rows q < l_b never read
k >= l_b, and rows q >= l_b are zeroed on the host). Every core processes
the same multiset of per-batch lengths, so one SPMD program serves all
cores with perfectly balanced load; only the W/bias column slices differ
per core, and all x tiles are broadcast.

Per-core pipeline (per batch slot, nl tiles of 128):
  - QK projection bf16, outputs [j, s] with j = q|k feature groups of the
    2 heads; PSUM->SBUF copy with per-partition bias add on DVE.
  - V projection bf16 in [j, s] orientation (weights stationary, x moving
    512 wide), bias via ones-row matmul; transposed back to [s, j] with
    the XBAR DMA-transpose; ones column appended for the softmax
    denominator.
  - Scores per head as K^T tile x Q chunks (64-deep contraction); exp on
    Act; causal mask of the diagonal tile multiplied on the idle GpSimd
    (Pool) engine (SBUF-only op).
  - AV with PSUM column packing: [q,65] slots for 4 t_q share one bank,
    col 64 accumulating the denominator via the V ones-column.
  - Normalize with one reciprocal + one stride0-broadcast tensor_tensor
    per 4-t_q group on DVE; bf16 output staged and DMA'd per batch.
"""

import sys

sys.path.insert(0, "/opt/trn_rl_repo")

import numpy as np
import ml_dtypes

import concourse.bass as bass
import concourse.bacc as bacc
import concourse.mybir as mybir
import concourse.tile as tile
from concourse.bass import ds, ts, broadcast_tensor_aps
from concourse.bass_utils import run_bass_kernel_spmd

P = 128
B, S, E, H = 8, 1024, 1024, 16
DH = E // H  # 64
NT = S // P  # 8
F32 = mybir.dt.float32
BF16 = mybir.dt.bfloat16

_cached = {}


def _build_program(nls):
    nc = bacc.Bacc(None, target_bir_lowering=False)

    xb = [nc.dram_tensor(f"xb_{i}", [P, NT, nl * P], BF16,
                         kind="ExternalInput")[:] for i, nl in enumerate(nls)]
    wqk = nc.dram_tensor("wqk", [P, NT, 2, P], BF16, kind="ExternalInput")[:]
    wv = nc.dram_tensor("wv", [P, NT, P], BF16, kind="ExternalInput")[:]
    bqk = nc.dram_tensor("bqk", [P, 2], F32, kind="ExternalInput")[:]
    bv = nc.dram_tensor("bv", [1, P], BF16, kind="ExternalInput")[:]
    cm = nc.dram_tensor("cm", [P, P], BF16, kind="ExternalInput")[:]
    ident = nc.dram_tensor("ident", [P, P], BF16, kind="ExternalInput")[:]
    ones = nc.dram_tensor("ones", [1, 512], BF16, kind="ExternalInput")[:]
    total = sum(nl * P for nl in nls)
    o = nc.dram_tensor("o", [total, P], BF16, kind="ExternalOutput")[:]

    with tile.TileContext(nc) as tc:
        from contextlib import ExitStack

        with ExitStack() as ctx:
            sb = ctx.enter_context(tc.tile_pool(name="sb", bufs=1))
            wqk_sb = sb.tile([P, NT, 2, P], BF16)
            wv_sb = sb.tile([P, NT, P], BF16)
            bqk_sb = sb.tile([P, 2], F32)
            bv_sb = sb.tile([1, P], BF16)
            cm_sb = sb.tile([P, P], BF16)
            ident_sb = sb.tile([P, P], BF16)
            ones_sb = sb.tile([1, 512], BF16)

            # ordered so the first QK matmul's inputs land first
            nc.sync.dma_start(out=wqk_sb[:, :, 0, :], in_=wqk[:, :, 0, :])

            xbp = ctx.enter_context(tc.tile_pool(name="xbp", bufs=2))

            def load_x(i, nl):
                # column-chunked so the first QK matmul starts early
                xbt = xbp.tile([P, NT, NT * P], BF16, name="xbt")
                for c0 in range(0, nl * P, 512):
                    cn = min(512, nl * P - c0)
                    nc.sync.dma_start(out=xbt[:, :, ds(c0, cn)],
                                      in_=xb[i][:, :, ds(c0, cn)])
                return xbt

            xtiles = {}

            def load_x0(nl):
                xbt = xbp.tile([P, NT, NT * P], BF16, name="xbt")
                cn0 = min(512, nl * P)
                nc.sync.dma_start(out=xbt[:, :, 0:cn0],
                                  in_=xb[0][:, :, 0:cn0])
                nc.sync.dma_start(out=wqk_sb[:, :, 1, :], in_=wqk[:, :, 1, :])
                for c0 in range(512, nl * P, 512):
                    cn = min(512, nl * P - c0)
                    nc.sync.dma_start(out=xbt[:, :, ds(c0, cn)],
                                      in_=xb[0][:, :, ds(c0, cn)])
                return xbt

            xtiles[0] = load_x0(nls[0])
            nc.sync.dma_start(out=wv_sb, in_=wv)
            for t, src in [(bqk_sb, bqk), (bv_sb, bv), (cm_sb, cm),
                           (ident_sb, ident), (ones_sb, ones)]:
                nc.sync.dma_start(out=t, in_=src)

            qkp = ctx.enter_context(tc.tile_pool(name="qkp", bufs=2))
            vtp = ctx.enter_context(tc.tile_pool(name="vtp", bufs=2))
            vpp = ctx.enter_context(tc.tile_pool(name="vpp", bufs=2))
            etp = ctx.enter_context(tc.tile_pool(name="etp", bufs=2))
            otp = ctx.enter_context(tc.tile_pool(name="otp", bufs=2))
            rcp = ctx.enter_context(tc.tile_pool(name="rcp", bufs=4))
            proj_ps = ctx.enter_context(
                tc.tile_pool(name="proj_ps", bufs=2, space="PSUM"))
            tr_ps = ctx.enter_context(
                tc.tile_pool(name="tr_ps", bufs=1, space="PSUM"))
            sc_ps = ctx.enter_context(
                tc.tile_pool(name="sc_ps", bufs=2, space="PSUM"))
            av_ps = ctx.enter_context(
                tc.tile_pool(name="av_ps", bufs=3, space="PSUM"))

            rowbase = 0
            for i, nl in enumerate(nls):
                Si = nl * P
                xbt = xtiles.pop(i)
                if i + 1 < len(nls):
                    xtiles[i + 1] = load_x(i + 1, nls[i + 1])

                # --- QK projection (bf16): psum [j, s-chunk], j = 2 heads
                qk = qkp.tile([P, 2, NT * P], BF16, name="qk")
                for c0 in range(0, Si, 512):
                    cn = min(512, Si - c0)
                    for g in range(2):
                        ps = proj_ps.tile([P, 512], F32)
                        for et in range(NT):
                            nc.tensor.matmul(
                                ps[:, 0:cn],
                                lhsT=wqk_sb[:, et, g, :],
                                rhs=xbt[:, et, ds(c0, cn)],
                                start=(et == 0), stop=(et == NT - 1))
                        nc.vector.tensor_scalar_add(
                            out=qk[:, g, ds(c0, cn)], in0=ps[:, 0:cn],
                            scalar1=bqk_sb[:, ds(g, 1)])

                # --- V projection (bf16) in [j, s], then XBAR transpose
                vT = vtp.tile([P, NT * P], BF16, name="vT")
                for c0 in range(0, Si, 512):
                    cn = min(512, Si - c0)
                    ps = proj_ps.tile([P, 512], F32)
                    for et in range(NT):
                        nc.tensor.matmul(
                            ps[:, 0:cn],
                            lhsT=wv_sb[:, et, :],
                            rhs=xbt[:, et, ds(c0, cn)],
                            start=(et == 0), stop=False)
                    nc.tensor.matmul(ps[:, 0:cn], lhsT=bv_sb,
                                     rhs=ones_sb[:, 0:cn],
                                     start=False, stop=True)
                    nc.vector.tensor_copy(out=vT[:, ds(c0, cn)],
                                          in_=ps[:, 0:cn])
                vp = vpp.tile([P, NT, 2, DH + 1], BF16, name="vp")
                nc.gpsimd.memset(vp[:, 0:nl, :, DH:DH + 1], 1.0)
                for st0 in range(0, nl, 4):
                    gs = min(4, nl - st0)
                    pt = tr_ps.tile([P, 4, P], BF16)
                    for st in range(st0, st0 + gs):
                        nc.tensor.transpose(
                            pt[:, st - st0, :], vT[:, ts(st, P)], ident_sb)
                    nc.vector.tensor_copy(
                        out=vp[:, ds(st0, gs), :, 0:DH],
                        in_=pt[:, 0:gs, :].rearrange(
                            "p t (h d) -> p t h d", h=2))

                # --- Scores + exp per head; fused diag mask on Pool.
                # eT rows padded to NT*P + P so all diagonal tiles sit at a
                # regular stride of 1280 columns in the flattened view.
                ets = []
                for h in range(2):
                    h0 = h * DH
                    eT = etp.tile([P, NT, NT * P + P], BF16, name="eT")
                    for t in range(nl):
                        c0 = t * P
                        while c0 < Si:
                            cn = min(512, Si - c0)
                            ps = sc_ps.tile([P, 512], F32)
                            nc.tensor.matmul(
                                ps[:, 0:cn],
                                lhsT=qk[h0:h0 + DH, 1, ts(t, P)],
                                rhs=qk[h0:h0 + DH, 0, ds(c0, cn)],
                                start=True, stop=True)
                            nc.scalar.activation(
                                out=eT[:, t, ds(c0, cn)], in_=ps[:, 0:cn],
                                func=mybir.ActivationFunctionType.Exp,
                                scale=1.0 / 32.0)
                            c0 += cn
                    flat = eT.rearrange("p a b -> p (a b)")
                    cmb = cm_sb.rearrange("p (o c) -> p o c", o=1)
                    if nl > 1:
                        dg = flat[:, 0:(nl - 1) * 1280].rearrange(
                            "p (n r) -> p n r", r=1280)[:, :, 0:P]
                        d0, d1 = broadcast_tensor_aps(dg, cmb)
                        nc.gpsimd.tensor_tensor(out=d0, in0=d0, in1=d1,
                                                op=mybir.AluOpType.mult)
                    last = flat[:, ds((nl - 1) * 1280, P)]
                    nc.gpsimd.tensor_mul(last, last, cm_sb)
                    ets.append(eT)

                # --- AV + normalize per head, 4 t_q per PSUM bank
                out_sb = otp.tile([P, NT, P], BF16, name="out_sb")
                for h in range(2):
                    eT = ets[h]
                    for tq0 in range(0, nl, 4):
                        g = min(4, nl - tq0)
                        po = av_ps.tile([P, 260], F32)
                        for tq in range(tq0, tq0 + g):
                            sl = tq - tq0
                            for tk in range(tq + 1):
                                nc.tensor.matmul(
                                    po[:, ds(sl * 65, DH + 1)],
                                    lhsT=eT[:, tk, ts(tq, P)],
                                    rhs=vp[:, tk, h, :],
                                    start=(tk == 0), stop=(tk == tq))
                        pot = po.rearrange("p (t c) -> p t c", c=65)
                        rec = rcp.tile([P, 4], F32, name="rec")
                        nc.vector.reciprocal(rec[:, 0:g], pot[:, 0:g, 64])
                        in0 = pot[:, 0:g, 0:DH]
                        in1 = rec[:, 0:g].rearrange("p (t o) -> p t o", o=1)
                        b0, b1 = broadcast_tensor_aps(in0, in1)
                        nc.vector.tensor_tensor(
                            out=out_sb[:, ds(tq0, g), ds(h * DH, DH)],
                            in0=b0, in1=b1, op=mybir.AluOpType.mult)

                nc.sync.dma_start(
                    out=o[ds(rowbase, Si), :].rearrange(
                        "(t p) c -> p t c", p=P),
                    in_=out_sb[:, 0:nl, :])
                rowbase += Si

    nc.compile()
    return nc


def _prepare(x, l, W, b):
    lv = np.asarray(l).astype(np.int64)
    nl = np.minimum((lv + P - 1) // P, NT).astype(np.int64)
    order = sorted(range(B), key=lambda i: -int(nl[i]))
    nls = tuple(int(nl[i]) for i in order)

    common = {}
    for i, bi in enumerate(order):
        n = nls[i]
        xT = np.ascontiguousarray(x[bi].T[:, 0:n * P])  # [E, n*128] f32
        xr = xT.reshape(NT, P, n * P)
        common[f"xb_{i}"] = np.ascontiguousarray(
            xr.transpose(1, 0, 2).astype(ml_dtypes.bfloat16))
    idx = np.arange(P)
    common["cm"] = np.ascontiguousarray(
        (idx[:, None] <= idx[None, :]).astype(ml_dtypes.bfloat16))
    common["ident"] = np.eye(P).astype(ml_dtypes.bfloat16)
    common["ones"] = np.ones((1, 512), ml_dtypes.bfloat16)

    in_maps = []
    for c in range(B):
        r0 = 2 * c * DH  # first feature row of this core's 2 heads
        wq = W[r0:r0 + P]             # [128, E]
        wk = W[E + r0:E + r0 + P]
        wvs = W[2 * E + r0:2 * E + r0 + P]
        wqk_c = np.stack([wq.T, wk.T], axis=1)      # [E, 2, 128]
        wqk_c = wqk_c.reshape(NT, P, 2, P).transpose(1, 0, 2, 3)
        wv_c = wvs.T.reshape(NT, P, P).transpose(1, 0, 2)
        m = dict(common)
        m["wqk"] = np.ascontiguousarray(wqk_c.astype(ml_dtypes.bfloat16))
        m["wv"] = np.ascontiguousarray(wv_c.astype(ml_dtypes.bfloat16))
        m["bqk"] = np.ascontiguousarray(
            np.stack([b[r0:r0 + P], b[E + r0:E + r0 + P]], axis=1)
            .astype(np.float32))
        m["bv"] = np.ascontiguousarray(
            b[2 * E + r0:2 * E + r0 + P].astype(ml_dtypes.bfloat16)
            .reshape(1, P))
        in_maps.append(m)
    return in_maps, order, nls


def _run(x, l, W, b, trace=False):
    x = np.asarray(x, dtype=np.float32)
    W = np.asarray(W, dtype=np.float32)
    b = np.asarray(b, dtype=np.float32)
    in_maps, order, nls = _prepare(x, l, W, b)
    if nls not in _cached:
        _cached[nls] = _build_program(nls)
    nc = _cached[nls]
    res = run_bass_kernel_spmd(nc, in_maps, list(range(B)), trace=trace)

    lv = np.asarray(l).astype(np.int64)
    out = np.zeros((B, S, E), dtype=np.float32)
    for c in range(B):
        oc = np.asarray(res.results[c]["o"]).astype(np.float32)
        rowbase = 0
        for i, bi in enumerate(order):
            n = nls[i]
            lb = int(lv[bi])
            rows = min(lb, n * P)
            out[bi, 0:rows, P * c:P * (c + 1)] = oc[rowbase:rowbase + rows]
            rowbase += n * P
    return out, res.exec_time_ns


def kernel(x, l, W, b):
    out, _ = _run(x, l, W, b, trace=False)
    return out


# revision 24
# speedup vs baseline: 1.9695x; 1.0326x over previous
"""Multi-head self-attention (B=8, S=1024, E=1024, H=16) on 8 TRN2 cores.

Sharding: head-parallel with length clipping. Core c owns heads {2c, 2c+1}
for ALL batches; each batch b is clipped to nl_b = ceil(l_b/128) tiles of
128 sequence positions (causal attention means rows q < l_b never read
k >= l_b, and rows q >= l_b are zeroed on the host). Every core processes
the same multiset of per-batch lengths, so one SPMD program serves all
cores with perfectly balanced load; only the W/bias column slices differ
per core, and all x tiles are broadcast.

Per-core pipeline (per batch slot, nl tiles of 128):
  - QK projection bf16, outputs [j, s] with j = q|k feature groups of the
    2 heads; PSUM->SBUF copy with per-partition bias add on DVE.
  - V projection bf16 in [j, s] orientation (weights stationary, x moving
    512 wide), per-partition bias folded into the PSUM->SBUF copy;
    transposed back to [s, j] with PE is_transpose matmuls; ones column
    appended for the softmax denominator.
  - Scores per head as K^T tile x Q chunks (64-deep contraction); exp on
    Act; causal mask of the diagonal tile multiplied on the idle GpSimd
    (Pool) engine (SBUF-only op).
  - AV with PSUM column packing: [q,65] slots for 4 t_q share one bank,
    col 64 accumulating the denominator via the V ones-column.
  - Normalize with one reciprocal + one stride0-broadcast tensor_tensor
    per 4-t_q group on DVE; bf16 output staged and DMA'd per batch.
"""

import sys

sys.path.insert(0, "/opt/trn_rl_repo")

import numpy as np
import ml_dtypes

import concourse.bass as bass
import concourse.bacc as bacc
import concourse.mybir as mybir
import concourse.tile as tile
from concourse.bass import ds, ts, broadcast_tensor_aps
from concourse.bass_utils import run_bass_kernel_spmd

P = 128
B, S, E, H = 8, 1024, 1024, 16
DH = E // H  # 64
NT = S // P  # 8
F32 = mybir.dt.float32
BF16 = mybir.dt.bfloat16

_cached = {}


def _build_program(nls):
    nc = bacc.Bacc(None, target_bir_lowering=False)

    xb = [nc.dram_tensor(f"xb_{i}", [P, NT, nl * P], BF16,
                         kind="ExternalInput")[:] for i, nl in enumerate(nls)]
    wqk = nc.dram_tensor("wqk", [P, NT, 2, P], BF16, kind="ExternalInput")[:]
    wv = nc.dram_tensor("wv", [P, NT, P], BF16, kind="ExternalInput")[:]
    bqk = nc.dram_tensor("bqk", [P, 2], F32, kind="ExternalInput")[:]
    bv = nc.dram_tensor("bv", [P, 1], F32, kind="ExternalInput")[:]
    cm = nc.dram_tensor("cm", [P, P], BF16, kind="ExternalInput")[:]
    ident = nc.dram_tensor("ident", [P, P], BF16, kind="ExternalInput")[:]
    total = sum(nl * P for nl in nls)
    o = nc.dram_tensor("o", [total, P], BF16, kind="ExternalOutput")[:]

    with tile.TileContext(nc) as tc:
        from contextlib import ExitStack

        with ExitStack() as ctx:
            sb = ctx.enter_context(tc.tile_pool(name="sb", bufs=1))
            wqk_sb = sb.tile([P, NT, 2, P], BF16)
            wv_sb = sb.tile([P, NT, P], BF16)
            bqk_sb = sb.tile([P, 2], F32)
            bv_sb = sb.tile([P, 1], F32)
            cm_sb = sb.tile([P, P], BF16)
            ident_sb = sb.tile([P, P], BF16)

            # ordered so the first QK matmul's inputs land first
            nc.sync.dma_start(out=wqk_sb[:, :, 0, :], in_=wqk[:, :, 0, :])

            xbp = ctx.enter_context(tc.tile_pool(name="xbp", bufs=2))

            def load_x(i, nl):
                # column-chunked so the first QK matmul starts early
                xbt = xbp.tile([P, NT, NT * P], BF16, name="xbt")
                for c0 in range(0, nl * P, 512):
                    cn = min(512, nl * P - c0)
                    nc.sync.dma_start(out=xbt[:, :, ds(c0, cn)],
                                      in_=xb[i][:, :, ds(c0, cn)])
                return xbt

            xtiles = {}

            def chunk_list(Si, first_small):
                cs, c0 = [], 0
                if first_small and Si >= 512:
                    cs, c0 = [(0, 256), (256, 256)], 512
                while c0 < Si:
                    cn = min(512, Si - c0)
                    cs.append((c0, cn))
                    c0 += cn
                return cs

            def load_x0(nl):
                xbt = xbp.tile([P, NT, NT * P], BF16, name="xbt")
                nc.sync.dma_start(out=bqk_sb, in_=bqk)
                cs = chunk_list(nl * P, True)
                first = True
                for c0, cn in cs:
                    nc.sync.dma_start(out=xbt[:, :, ds(c0, cn)],
                                      in_=xb[0][:, :, ds(c0, cn)])
                    if first:
                        nc.sync.dma_start(out=wqk_sb[:, :, 1, :],
                                          in_=wqk[:, :, 1, :])
                        first = False
                return xbt

            xtiles[0] = load_x0(nls[0])
            nc.sync.dma_start(out=wv_sb, in_=wv)
            for t, src in [(bv_sb, bv), (cm_sb, cm),
                           (ident_sb, ident)]:
                nc.sync.dma_start(out=t, in_=src)

            qkp = ctx.enter_context(tc.tile_pool(name="qkp", bufs=2))
            vtp = ctx.enter_context(tc.tile_pool(name="vtp", bufs=2))
            vpp = ctx.enter_context(tc.tile_pool(name="vpp", bufs=2))
            etp = ctx.enter_context(tc.tile_pool(name="etp", bufs=2))
            otp = ctx.enter_context(tc.tile_pool(name="otp", bufs=2))
            rcp = ctx.enter_context(tc.tile_pool(name="rcp", bufs=4))
            proj_ps = ctx.enter_context(
                tc.tile_pool(name="proj_ps", bufs=2, space="PSUM"))
            tr_ps = ctx.enter_context(
                tc.tile_pool(name="tr_ps", bufs=1, space="PSUM"))
            sc_ps = ctx.enter_context(
                tc.tile_pool(name="sc_ps", bufs=2, space="PSUM"))
            av_ps = ctx.enter_context(
                tc.tile_pool(name="av_ps", bufs=3, space="PSUM"))

            rowbase = 0
            for i, nl in enumerate(nls):
                Si = nl * P
                xbt = xtiles.pop(i)
                if i + 1 < len(nls):
                    xtiles[i + 1] = load_x(i + 1, nls[i + 1])

                # --- QK projection (bf16): psum [j, s-chunk], j = 2 heads
                qk = qkp.tile([P, 2, NT * P], BF16, name="qk")
                for c0, cn in chunk_list(Si, i == 0):
                    for g in range(2):
                        ps = proj_ps.tile([P, 512], F32)
                        for et in range(NT):
                            nc.tensor.matmul(
                                ps[:, 0:cn],
                                lhsT=wqk_sb[:, et, g, :],
                                rhs=xbt[:, et, ds(c0, cn)],
                                start=(et == 0), stop=(et == NT - 1))
                        nc.vector.tensor_scalar_add(
                            out=qk[:, g, ds(c0, cn)], in0=ps[:, 0:cn],
                            scalar1=bqk_sb[:, ds(g, 1)])

                # --- V projection (bf16) in [j, s], then XBAR transpose
                vT = vtp.tile([P, NT * P], BF16, name="vT")
                for c0 in range(0, Si, 512):
                    cn = min(512, Si - c0)
                    ps = proj_ps.tile([P, 512], F32)
                    for et in range(NT):
                        nc.tensor.matmul(
                            ps[:, 0:cn],
                            lhsT=wv_sb[:, et, :],
                            rhs=xbt[:, et, ds(c0, cn)],
                            start=(et == 0), stop=(et == NT - 1))
                    nc.vector.tensor_scalar_add(
                        out=vT[:, ds(c0, cn)], in0=ps[:, 0:cn],
                        scalar1=bv_sb)
                vp = vpp.tile([P, NT, 2, DH + 1], BF16, name="vp")
                nc.gpsimd.memset(vp[:, 0:nl, :, DH:DH + 1], 1.0)
                for st0 in range(0, nl, 4):
                    gs = min(4, nl - st0)
                    pt = tr_ps.tile([P, 4, P], BF16)
                    for st in range(st0, st0 + gs):
                        nc.tensor.transpose(
                            pt[:, st - st0, :], vT[:, ts(st, P)], ident_sb)
                    nc.vector.tensor_copy(
                        out=vp[:, ds(st0, gs), :, 0:DH],
                        in_=pt[:, 0:gs, :].rearrange(
                            "p t (h d) -> p t h d", h=2))

                # --- Scores + exp per head; fused diag mask on Pool.
                # eT rows padded to NT*P + P so all diagonal tiles sit at a
                # regular stride of 1280 columns in the flattened view.
                ets = []
                for h in range(2):
                    h0 = h * DH
                    eT = etp.tile([P, NT, NT * P + P], BF16, name="eT")
                    for t in range(nl):
                        c0 = t * P
                        while c0 < Si:
                            cn = min(512, Si - c0)
                            ps = sc_ps.tile([P, 512], F32)
                            nc.tensor.matmul(
                                ps[:, 0:cn],
                                lhsT=qk[h0:h0 + DH, 1, ts(t, P)],
                                rhs=qk[h0:h0 + DH, 0, ds(c0, cn)],
                                start=True, stop=True)
                            nc.scalar.activation(
                                out=eT[:, t, ds(c0, cn)], in_=ps[:, 0:cn],
                                func=mybir.ActivationFunctionType.Exp,
                                scale=1.0 / 32.0)
                            c0 += cn
                    flat = eT.rearrange("p a b -> p (a b)")
                    cmb = cm_sb.rearrange("p (o c) -> p o c", o=1)
                    if nl > 1:
                        dg = flat[:, 0:(nl - 1) * 1280].rearrange(
                            "p (n r) -> p n r", r=1280)[:, :, 0:P]
                        d0, d1 = broadcast_tensor_aps(dg, cmb)
                        nc.gpsimd.tensor_tensor(out=d0, in0=d0, in1=d1,
                                                op=mybir.AluOpType.mult)
                    last = flat[:, ds((nl - 1) * 1280, P)]
                    nc.gpsimd.tensor_mul(last, last, cm_sb)
                    ets.append(eT)

                # --- AV + normalize per head, 4 t_q per PSUM bank
                out_sb = otp.tile([P, NT, P], BF16, name="out_sb")
                for h in range(2):
                    eT = ets[h]
                    for tq0 in range(0, nl, 4):
                        g = min(4, nl - tq0)
                        po = av_ps.tile([P, 260], F32)
                        for tq in range(tq0, tq0 + g):
                            sl = tq - tq0
                            for tk in range(tq + 1):
                                nc.tensor.matmul(
                                    po[:, ds(sl * 65, DH + 1)],
                                    lhsT=eT[:, tk, ts(tq, P)],
                                    rhs=vp[:, tk, h, :],
                                    start=(tk == 0), stop=(tk == tq))
                        pot = po.rearrange("p (t c) -> p t c", c=65)
                        rec = rcp.tile([P, 4], F32, name="rec")
                        nc.vector.reciprocal(rec[:, 0:g], pot[:, 0:g, 64])
                        in0 = pot[:, 0:g, 0:DH]
                        in1 = rec[:, 0:g].rearrange("p (t o) -> p t o", o=1)
                        b0, b1 = broadcast_tensor_aps(in0, in1)
                        nc.vector.tensor_tensor(
                            out=out_sb[:, ds(tq0, g), ds(h * DH, DH)],
                            in0=b0, in1=b1, op=mybir.AluOpType.mult)

                nc.sync.dma_start(
                    out=o[ds(rowbase, Si), :].rearrange(
                        "(t p) c -> p t c", p=P),
                    in_=out_sb[:, 0:nl, :])
                rowbase += Si

    nc.compile()
    return nc


def _prepare(x, l, W, b):
    lv = np.asarray(l).astype(np.int64)
    nl = np.minimum((lv + P - 1) // P, NT).astype(np.int64)
    order = sorted(range(B), key=lambda i: -int(nl[i]))
    nls = tuple(int(nl[i]) for i in order)

    common = {}
    for i, bi in enumerate(order):
        n = nls[i]
        xT = np.ascontiguousarray(x[bi].T[:, 0:n * P])  # [E, n*128] f32
        xr = xT.reshape(NT, P, n * P)
        common[f"xb_{i}"] = np.ascontiguousarray(
            xr.transpose(1, 0, 2).astype(ml_dtypes.bfloat16))
    idx = np.arange(P)
    common["cm"] = np.ascontiguousarray(
        (idx[:, None] <= idx[None, :]).astype(ml_dtypes.bfloat16))
    common["ident"] = np.eye(P).astype(ml_dtypes.bfloat16)

    in_maps = []
    for c in range(B):
        r0 = 2 * c * DH  # first feature row of this core's 2 heads
        wq = W[r0:r0 + P]             # [128, E]
        wk = W[E + r0:E + r0 + P]
        wvs = W[2 * E + r0:2 * E + r0 + P]
        wqk_c = np.stack([wq.T, wk.T], axis=1)      # [E, 2, 128]
        wqk_c = wqk_c.reshape(NT, P, 2, P).transpose(1, 0, 2, 3)
        wv_c = wvs.T.reshape(NT, P, P).transpose(1, 0, 2)
        m = dict(common)
        m["wqk"] = np.ascontiguousarray(wqk_c.astype(ml_dtypes.bfloat16))
        m["wv"] = np.ascontiguousarray(wv_c.astype(ml_dtypes.bfloat16))
        m["bqk"] = np.ascontiguousarray(
            np.stack([b[r0:r0 + P], b[E + r0:E + r0 + P]], axis=1)
            .astype(np.float32))
        m["bv"] = np.ascontiguousarray(
            b[2 * E + r0:2 * E + r0 + P].astype(np.float32).reshape(P, 1))
        in_maps.append(m)
    return in_maps, order, nls


def _run(x, l, W, b, trace=False):
    x = np.asarray(x, dtype=np.float32)
    W = np.asarray(W, dtype=np.float32)
    b = np.asarray(b, dtype=np.float32)
    in_maps, order, nls = _prepare(x, l, W, b)
    if nls not in _cached:
        _cached[nls] = _build_program(nls)
    nc = _cached[nls]
    res = run_bass_kernel_spmd(nc, in_maps, list(range(B)), trace=trace)

    lv = np.asarray(l).astype(np.int64)
    out = np.zeros((B, S, E), dtype=np.float32)
    for c in range(B):
        oc = np.asarray(res.results[c]["o"]).astype(np.float32)
        rowbase = 0
        for i, bi in enumerate(order):
            n = nls[i]
            lb = int(lv[bi])
            rows = min(lb, n * P)
            out[bi, 0:rows, P * c:P * (c + 1)] = oc[rowbase:rowbase + rows]
            rowbase += n * P
    return out, res.exec_time_ns


def kernel(x, l, W, b):
    out, _ = _run(x, l, W, b, trace=False)
    return out


# revision 25
# speedup vs baseline: 1.9841x; 1.0074x over previous
"""Multi-head self-attention (B=8, S=1024, E=1024, H=16) on 8 TRN2 cores.

Sharding: head-parallel with length clipping. Core c owns heads {2c, 2c+1}
for ALL batches; each batch b is clipped to nl_b = ceil(l_b/128) tiles of
128 sequence positions (causal attention means rows q < l_b never read
k >= l_b, and rows q >= l_b are zeroed on the host). Every core processes
the same multiset of per-batch lengths, so one SPMD program serves all
cores with perfectly balanced load; only the W/bias column slices differ
per core, and all x tiles are broadcast.

Per-core pipeline (per batch slot, nl tiles of 128):
  - QK projection bf16, outputs [j, s] with j = q|k feature groups of the
    2 heads; PSUM->SBUF copy with per-partition bias add on DVE.
  - V projection bf16 in [j, s] orientation (weights stationary, x moving
    512 wide), per-partition bias folded into the PSUM->SBUF copy;
    transposed back to [s, j] with PE is_transpose matmuls; ones column
    appended for the softmax denominator.
  - Scores per head as K^T tile x Q chunks (64-deep contraction); exp on
    Act; causal mask of the diagonal tile multiplied on the idle GpSimd
    (Pool) engine (SBUF-only op).
  - AV with PSUM column packing: [q,65] slots for 4 t_q share one bank,
    col 64 accumulating the denominator via the V ones-column.
  - Normalize with one reciprocal + one stride0-broadcast tensor_tensor
    per 4-t_q group on DVE; bf16 output staged and DMA'd per batch.
"""

import sys

sys.path.insert(0, "/opt/trn_rl_repo")

import numpy as np
import ml_dtypes

import concourse.bass as bass
import concourse.bacc as bacc
import concourse.mybir as mybir
import concourse.tile as tile
from concourse.bass import ds, ts, broadcast_tensor_aps
from concourse.bass_utils import run_bass_kernel_spmd

P = 128
B, S, E, H = 8, 1024, 1024, 16
DH = E // H  # 64
NT = S // P  # 8
F32 = mybir.dt.float32
BF16 = mybir.dt.bfloat16

_cached = {}


def _build_program(nls):
    nc = bacc.Bacc(None, target_bir_lowering=False)

    xb = [nc.dram_tensor(f"xb_{i}", [P, NT, nl * P], BF16,
                         kind="ExternalInput")[:] for i, nl in enumerate(nls)]
    wqk = nc.dram_tensor("wqk", [P, NT, 2, P], BF16, kind="ExternalInput")[:]
    wv = nc.dram_tensor("wv", [P, NT, P], BF16, kind="ExternalInput")[:]
    bqk = nc.dram_tensor("bqk", [P, 2], F32, kind="ExternalInput")[:]
    bv = nc.dram_tensor("bv", [P, 1], F32, kind="ExternalInput")[:]
    cm = nc.dram_tensor("cm", [P, P], BF16, kind="ExternalInput")[:]
    ident = nc.dram_tensor("ident", [P, P], BF16, kind="ExternalInput")[:]
    total = sum(nl * P for nl in nls)
    o = nc.dram_tensor("o", [total, P], BF16, kind="ExternalOutput")[:]

    with tile.TileContext(nc) as tc:
        from contextlib import ExitStack

        with ExitStack() as ctx:
            sb = ctx.enter_context(tc.tile_pool(name="sb", bufs=1))
            wqk_sb = sb.tile([P, NT, 2, P], BF16)
            wv_sb = sb.tile([P, NT, P], BF16)
            bqk_sb = sb.tile([P, 2], F32)
            bv_sb = sb.tile([P, 1], F32)
            cm_sb = sb.tile([P, P], BF16)
            ident_sb = sb.tile([P, P], BF16)

            # ordered so the first QK matmul's inputs land first
            nc.sync.dma_start(out=wqk_sb[:, :, 0, :], in_=wqk[:, :, 0, :])

            xbp = ctx.enter_context(tc.tile_pool(name="xbp", bufs=2))

            def load_x(i, nl):
                # column-chunked so the first QK matmul starts early
                xbt = xbp.tile([P, NT, NT * P], BF16, name="xbt")
                for c0 in range(0, nl * P, 512):
                    cn = min(512, nl * P - c0)
                    nc.sync.dma_start(out=xbt[:, :, ds(c0, cn)],
                                      in_=xb[i][:, :, ds(c0, cn)])
                return xbt

            xtiles = {}

            def chunk_list(Si, first_small):
                cs, c0 = [], 0
                if first_small and Si >= 512:
                    cs, c0 = [(0, 256), (256, 256)], 512
                while c0 < Si:
                    cn = min(512, Si - c0)
                    cs.append((c0, cn))
                    c0 += cn
                return cs

            def load_x0(nl):
                xbt = xbp.tile([P, NT, NT * P], BF16, name="xbt")
                nc.sync.dma_start(out=bqk_sb, in_=bqk)
                cs = chunk_list(nl * P, True)
                first = True
                for c0, cn in cs:
                    nc.sync.dma_start(out=xbt[:, :, ds(c0, cn)],
                                      in_=xb[0][:, :, ds(c0, cn)])
                    if first:
                        nc.sync.dma_start(out=wqk_sb[:, :, 1, :],
                                          in_=wqk[:, :, 1, :])
                        first = False
                return xbt

            xtiles[0] = load_x0(nls[0])
            nc.sync.dma_start(out=wv_sb, in_=wv)
            for t, src in [(bv_sb, bv), (cm_sb, cm),
                           (ident_sb, ident)]:
                nc.sync.dma_start(out=t, in_=src)

            qkp = ctx.enter_context(tc.tile_pool(name="qkp", bufs=2))
            vtp = ctx.enter_context(tc.tile_pool(name="vtp", bufs=2))
            vpp = ctx.enter_context(tc.tile_pool(name="vpp", bufs=2))
            etp = ctx.enter_context(tc.tile_pool(name="etp", bufs=2))
            otp = ctx.enter_context(tc.tile_pool(name="otp", bufs=2))
            rcp = ctx.enter_context(tc.tile_pool(name="rcp", bufs=4))
            proj_ps = ctx.enter_context(
                tc.tile_pool(name="proj_ps", bufs=2, space="PSUM"))
            tr_ps = ctx.enter_context(
                tc.tile_pool(name="tr_ps", bufs=1, space="PSUM"))
            sc_ps = ctx.enter_context(
                tc.tile_pool(name="sc_ps", bufs=2, space="PSUM"))
            av_ps = ctx.enter_context(
                tc.tile_pool(name="av_ps", bufs=3, space="PSUM"))

            def interleave(primary, filler, lead=0):
                fi = 0
                for _ in range(min(lead, len(filler))):
                    filler[fi]()
                    fi += 1
                nf = len(filler)
                for j, p in enumerate(primary):
                    p()
                    tgt = min(nf, lead + (j + 1) * nf // max(len(primary), 1))
                    while fi < tgt:
                        filler[fi]()
                        fi += 1
                while fi < nf:
                    filler[fi]()
                    fi += 1

            def emit_qk(i, nl, xbt):
                # returns (qk tile, list of per-chunk emitter closures)
                qk = qkp.tile([P, 2, NT * P], BF16, name="qk")
                ems = []
                for c0, cn in chunk_list(nl * P, i == 0):
                    for g in range(2):
                        def em(c0=c0, cn=cn, g=g):
                            ps = proj_ps.tile([P, 512], F32)
                            for et in range(NT):
                                nc.tensor.matmul(
                                    ps[:, 0:cn],
                                    lhsT=wqk_sb[:, et, g, :],
                                    rhs=xbt[:, et, ds(c0, cn)],
                                    start=(et == 0), stop=(et == NT - 1))
                            nc.vector.tensor_scalar_add(
                                out=qk[:, g, ds(c0, cn)], in0=ps[:, 0:cn],
                                scalar1=bqk_sb[:, ds(g, 1)])
                        ems.append(em)
                return qk, ems

            rowbase = 0
            qk_cur, qk_ems = emit_qk(0, nls[0], xtiles[0])
            for em in qk_ems:
                em()
            for i, nl in enumerate(nls):
                Si = nl * P
                xbt = xtiles.pop(i)
                if i + 1 < len(nls):
                    xtiles[i + 1] = load_x(i + 1, nls[i + 1])
                qk = qk_cur

                # --- V projection emitters: [j, s] chunks + PE transposes
                vT = vtp.tile([P, NT * P], BF16, name="vT")
                vp = vpp.tile([P, NT, 2, DH + 1], BF16, name="vp")
                nc.gpsimd.memset(vp[:, 0:nl, :, DH:DH + 1], 1.0)
                vfill = []
                for c0 in range(0, Si, 512):
                    cn = min(512, Si - c0)

                    def vem(c0=c0, cn=cn):
                        ps = proj_ps.tile([P, 512], F32)
                        for et in range(NT):
                            nc.tensor.matmul(
                                ps[:, 0:cn],
                                lhsT=wv_sb[:, et, :],
                                rhs=xbt[:, et, ds(c0, cn)],
                                start=(et == 0), stop=(et == NT - 1))
                        nc.vector.tensor_scalar_add(
                            out=vT[:, ds(c0, cn)], in0=ps[:, 0:cn],
                            scalar1=bv_sb)
                    vfill.append(vem)
                for st0 in range(0, nl, 4):
                    gs = min(4, nl - st0)

                    def tem(st0=st0, gs=gs):
                        pt = tr_ps.tile([P, 4, P], BF16)
                        for st in range(st0, st0 + gs):
                            nc.tensor.transpose(
                                pt[:, st - st0, :], vT[:, ts(st, P)],
                                ident_sb)
                        nc.vector.tensor_copy(
                            out=vp[:, ds(st0, gs), :, 0:DH],
                            in_=pt[:, 0:gs, :].rearrange(
                                "p t (h d) -> p t h d", h=2))
                    vfill.append(tem)

                # --- Score emitters + fused diag mask per head
                def make_scores(h, eT):
                    h0 = h * DH
                    ems = []
                    for t in range(nl):
                        c0 = t * P
                        while c0 < Si:
                            cn = min(512, Si - c0)

                            def sem(t=t, c0=c0, cn=cn):
                                ps = sc_ps.tile([P, 512], F32)
                                nc.tensor.matmul(
                                    ps[:, 0:cn],
                                    lhsT=qk[h0:h0 + DH, 1, ts(t, P)],
                                    rhs=qk[h0:h0 + DH, 0, ds(c0, cn)],
                                    start=True, stop=True)
                                nc.scalar.activation(
                                    out=eT[:, t, ds(c0, cn)],
                                    in_=ps[:, 0:cn],
                                    func=mybir.ActivationFunctionType.Exp,
                                    scale=1.0 / 32.0)
                            ems.append(sem)
                            c0 += cn
                    return ems

                def emit_mask(eT):
                    flat = eT.rearrange("p a b -> p (a b)")
                    cmb = cm_sb.rearrange("p (o c) -> p o c", o=1)
                    if nl > 1:
                        dg = flat[:, 0:(nl - 1) * 1280].rearrange(
                            "p (n r) -> p n r", r=1280)[:, :, 0:P]
                        d0, d1 = broadcast_tensor_aps(dg, cmb)
                        nc.gpsimd.tensor_tensor(out=d0, in0=d0, in1=d1,
                                                op=mybir.AluOpType.mult)
                    last = flat[:, ds((nl - 1) * 1280, P)]
                    nc.gpsimd.tensor_mul(last, last, cm_sb)

                # --- AV + normalize emitters per head
                out_sb = otp.tile([P, NT, P], BF16, name="out_sb")

                def make_av(h, eT):
                    ems = []
                    for tq0 in range(0, nl, 4):
                        g = min(4, nl - tq0)

                        def aem(tq0=tq0, g=g):
                            po = av_ps.tile([P, 260], F32)
                            for tq in range(tq0, tq0 + g):
                                sl = tq - tq0
                                for tk in range(tq + 1):
                                    nc.tensor.matmul(
                                        po[:, ds(sl * 65, DH + 1)],
                                        lhsT=eT[:, tk, ts(tq, P)],
                                        rhs=vp[:, tk, h, :],
                                        start=(tk == 0), stop=(tk == tq))
                            pot = po.rearrange("p (t c) -> p t c", c=65)
                            rec = rcp.tile([P, 4], F32, name="rec")
                            nc.vector.reciprocal(rec[:, 0:g], pot[:, 0:g, 64])
                            in0 = pot[:, 0:g, 0:DH]
                            in1 = rec[:, 0:g].rearrange(
                                "p (t o) -> p t o", o=1)
                            b0, b1 = broadcast_tensor_aps(in0, in1)
                            nc.vector.tensor_tensor(
                                out=out_sb[:, ds(tq0, g), ds(h * DH, DH)],
                                in0=b0, in1=b1, op=mybir.AluOpType.mult)
                        ems.append(aem)
                    return ems

                eT0 = etp.tile([P, NT, NT * P + P], BF16, name="eT")
                eT1 = etp.tile([P, NT, NT * P + P], BF16, name="eT")
                sc0 = make_scores(0, eT0)
                sc1 = make_scores(1, eT1)
                interleave(sc0, vfill, lead=1)
                emit_mask(eT0)
                av0 = make_av(0, eT0)
                interleave(sc1, av0)
                emit_mask(eT1)
                av1 = make_av(1, eT1)
                if i + 1 < len(nls):
                    qk_cur, qk_ems = emit_qk(i + 1, nls[i + 1],
                                             xtiles[i + 1])
                else:
                    qk_ems = []
                interleave(av1, qk_ems, lead=2)

                nc.sync.dma_start(
                    out=o[ds(rowbase, Si), :].rearrange(
                        "(t p) c -> p t c", p=P),
                    in_=out_sb[:, 0:nl, :])
                rowbase += Si

    nc.compile()
    return nc


def _prepare(x, l, W, b):
    lv = np.asarray(l).astype(np.int64)
    nl = np.minimum((lv + P - 1) // P, NT).astype(np.int64)
    order = sorted(range(B), key=lambda i: -int(nl[i]))
    nls = tuple(int(nl[i]) for i in order)

    common = {}
    for i, bi in enumerate(order):
        n = nls[i]
        xT = np.ascontiguousarray(x[bi].T[:, 0:n * P])  # [E, n*128] f32
        xr = xT.reshape(NT, P, n * P)
        common[f"xb_{i}"] = np.ascontiguousarray(
            xr.transpose(1, 0, 2).astype(ml_dtypes.bfloat16))
    idx = np.arange(P)
    common["cm"] = np.ascontiguousarray(
        (idx[:, None] <= idx[None, :]).astype(ml_dtypes.bfloat16))
    common["ident"] = np.eye(P).astype(ml_dtypes.bfloat16)

    in_maps = []
    for c in range(B):
        r0 = 2 * c * DH  # first feature row of this core's 2 heads
        wq = W[r0:r0 + P]             # [128, E]
        wk = W[E + r0:E + r0 + P]
        wvs = W[2 * E + r0:2 * E + r0 + P]
        wqk_c = np.stack([wq.T, wk.T], axis=1)      # [E, 2, 128]
        wqk_c = wqk_c.reshape(NT, P, 2, P).transpose(1, 0, 2, 3)
        wv_c = wvs.T.reshape(NT, P, P).transpose(1, 0, 2)
        m = dict(common)
        m["wqk"] = np.ascontiguousarray(wqk_c.astype(ml_dtypes.bfloat16))
        m["wv"] = np.ascontiguousarray(wv_c.astype(ml_dtypes.bfloat16))
        m["bqk"] = np.ascontiguousarray(
            np.stack([b[r0:r0 + P], b[E + r0:E + r0 + P]], axis=1)
            .astype(np.float32))
        m["bv"] = np.ascontiguousarray(
            b[2 * E + r0:2 * E + r0 + P].astype(np.float32).reshape(P, 1))
        in_maps.append(m)
    return in_maps, order, nls


def _run(x, l, W, b, trace=False):
    x = np.asarray(x, dtype=np.float32)
    W = np.asarray(W, dtype=np.float32)
    b = np.asarray(b, dtype=np.float32)
    in_maps, order, nls = _prepare(x, l, W, b)
    if nls not in _cached:
        _cached[nls] = _build_program(nls)
    nc = _cached[nls]
    res = run_bass_kernel_spmd(nc, in_maps, list(range(B)), trace=trace)

    lv = np.asarray(l).astype(np.int64)
    out = np.zeros((B, S, E), dtype=np.float32)
    for c in range(B):
        oc = np.asarray(res.results[c]["o"]).astype(np.float32)
        rowbase = 0
        for i, bi in enumerate(order):
            n = nls[i]
            lb = int(lv[bi])
            rows = min(lb, n * P)
            out[bi, 0:rows, P * c:P * (c + 1)] = oc[rowbase:rowbase + rows]
            rowbase += n * P
    return out, res.exec_time_ns


def kernel(x, l, W, b):
    out, _ = _run(x, l, W, b, trace=False)
    return out
